# revision 1
# baseline (speedup 1.0000x reference)
"""DAGCN kernel for Trainium2, 8 NeuronCores, sharded over the T (time) axis.

Math (per time step t):
  A      = relu(E_t E_t^T)            (outer product of embeddings, symmetric)
  PU     = exp(A)                     (unnormalized softmax numerator; A <= ~16 so no overflow,
                                       and exp(A - mx)/sum == exp(A)/sum_u exactly)
  inv[n] = 1 / sum_s PU[n, s]
  S      = diag(inv) PU               (the softmax scores)
  d[n]   = S[n, n] = exp(E[n]^2) * inv[n]
  xg0    = x
  xg1    = S @ x        = diag(inv) (PU @ x)
  xg2    = 2 d * xg1 - x
  out[b,n,o] = sum_i xg0[b,n,i] W0[n,i,o] + xg1 W1 + xg2 W2 + bias[n,o]

Per-core layout strategy:
  - e1 (PU @ x): PU tiles [m, n] (symmetric) serve directly as lhsT; x as [m, (b c)].
    fp32r matmuls (1 cyc/row at 512-wide moving operand).
  - The per-n second contraction runs with contraction dim c on partitions:
    stationary = W[n] tiles ([k0 i | k1 i] stacked to 128 rows + separate k2 [64]),
    moving = transposed xg stacks [c, b] built with PE transposes.
  - Output produced as [o, b] per n, PE-transposed back to [n-part, b, o] for a
    clean 3-dim DMA to HBM.
"""
import sys

sys.path.insert(0, "/opt/trn_rl_repo")
import numpy as np

B, T, N, C, O, K = 32, 24, 512, 64, 64, 3
NCORES = 8
T_LOC = T // NCORES  # 3 time steps per core

_CACHE = {}


def build_bass():
    if "nc" in _CACHE:
        return _CACHE["nc"]
    from contextlib import ExitStack

    import concourse.bass as bass
    import concourse.mybir as mybir
    from concourse import bacc
    import concourse.tile as tile
    from concourse.bass import ts
    from concourse.masks import make_identity

    f32 = mybir.dt.float32
    f32r = mybir.dt.float32r
    bf16 = mybir.dt.bfloat16
    Alu = mybir.AluOpType
    Act = mybir.ActivationFunctionType
    AX = mybir.AxisListType.X

    nc = bacc.Bacc()
    x_d = nc.dram_tensor("x_sh", [B, T_LOC, N, C], f32, kind="ExternalInput")
    e_d = nc.dram_tensor("emb_sh", [T_LOC, N], f32, kind="ExternalInput")
    w_d = nc.dram_tensor("w_sh", [T_LOC, N, K, C, O], f32, kind="ExternalInput")
    b_d = nc.dram_tensor("bias_sh", [T_LOC, N, O], f32, kind="ExternalInput")
    o_d = nc.dram_tensor("out_sh", [B, T_LOC, N, O], f32, kind="ExternalOutput")

    with tile.TileContext(nc) as tc, ExitStack() as ctx:
        p1 = ctx.enter_context(tc.tile_pool(name="singles", bufs=1))
        p_x = ctx.enter_context(tc.tile_pool(name="xin", bufs=5))
        p_ptu = ctx.enter_context(tc.tile_pool(name="ptu", bufs=5))
        p_sm = ctx.enter_context(tc.tile_pool(name="sm", bufs=6))
        p_xg = ctx.enter_context(tc.tile_pool(name="xg", bufs=3))
        p_stk = ctx.enter_context(tc.tile_pool(name="stk", bufs=1))
        p_w = ctx.enter_context(tc.tile_pool(name="wt", bufs=4))
        p_bt = ctx.enter_context(tc.tile_pool(name="bt", bufs=2))
        p_ot = ctx.enter_context(tc.tile_pool(name="ot", bufs=1))
        p_ob = ctx.enter_context(tc.tile_pool(name="ob", bufs=2))
        p_ps = ctx.enter_context(tc.tile_pool(name="ps", bufs=2, space="PSUM"))
        p_pst = ctx.enter_context(tc.tile_pool(name="pst", bufs=3, space="PSUM"))
        p_psb = ctx.enter_context(tc.tile_pool(name="psb", bufs=3, space="PSUM"))

        ident = p1.tile([128, 128], f32)
        make_identity(nc, ident)

        for t in range(T_LOC):
            # ---------------- softmax / scores phase ----------------
            e_row = p_sm.tile([1, 512], f32, tag="erow")
            nc.sync.dma_start(out=e_row, in_=e_d[t][None, :])
            e_col = p_sm.tile([128, 4], f32, tag="ecol")
            nc.sync.dma_start(out=e_col, in_=e_d[t].rearrange("(c p) -> p c", p=128))

            ptus = []
            for mc in range(4):
                ps = p_psb.tile([128, 512], f32, tag="big")
                nc.tensor.matmul(ps[:], e_row[:, ts(mc, 128)], e_row[:],
                                 start=True, stop=True)
                a_sb = p_xg.tile([128, 512], f32, tag="a")
                nc.vector.tensor_single_scalar(a_sb[:], ps[:], 0.0, Alu.max)  # relu
                ptu = p_ptu.tile([128, 512], f32r, tag="ptu")
                nc.scalar.activation(ptu[:], a_sb[:], Act.Exp)
                ptus.append(ptu)

            invs, s2dis = [], []
            for ci in range(4):
                rs = p_sm.tile([128, 1], f32, tag="rs")
                nc.vector.tensor_reduce(rs[:], ptus[ci][:], axis=AX, op=Alu.add)
                inv = p_sm.tile([128, 1], f32, tag="inv")
                nc.vector.reciprocal(inv[:], rs[:])
                sq = p_sm.tile([128, 1], f32, tag="sq")
                nc.vector.tensor_mul(sq[:], e_col[:, ci:ci + 1], e_col[:, ci:ci + 1])
                esq = p_sm.tile([128, 1], f32, tag="esq")
                nc.scalar.activation(esq[:], sq[:], Act.Exp)
                t1 = p_sm.tile([128, 1], f32, tag="t1")
                nc.vector.tensor_mul(t1[:], esq[:], inv[:])
                t2 = p_sm.tile([128, 1], f32, tag="t2")
                nc.vector.tensor_mul(t2[:], t1[:], inv[:])
                s2di = p_sm.tile([128, 1], f32, tag="s2di")
                nc.vector.tensor_add(s2di[:], t2[:], t2[:])  # x2
                invs.append(inv)
                s2dis.append(s2di)

            # ---------------- x load (whole t) ----------------
            xts, xrs = [], []
            for mc in range(4):
                xt = p_x.tile([128, 32, 64], f32, tag="x")
                nc.scalar.dma_start(
                    out=xt, in_=x_d[:, t, ts(mc, 128), :].rearrange("b m c -> m b c"))
                xts.append(xt)
                xr = p_x.tile([128, 2048], f32r, tag="xr")
                nc.gpsimd.tensor_copy(out=xr[:], in_=xt.rearrange("p b c -> p (b c)"))
                xrs.append(xr)

            # ---------------- per n-chunk pipeline ----------------
            for ci in range(4):
                stack1 = p_stk.tile([128, 32, 128], bf16, tag="s1")
                stack2 = p_stk.tile([64, 32, 128], bf16, tag="s2")
                for f in range(4):  # 8 b's per f
                    ps = p_psb.tile([128, 512], f32, tag="big")
                    for mc in range(4):
                        nc.tensor.matmul(
                            ps[:],
                            ptus[mc][:, ts(ci, 128)],
                            xrs[mc][:, ts(f, 512)],
                            start=(mc == 0), stop=(mc == 3))
                    xg1 = p_xg.tile([128, 512], f32, tag="xg1")
                    nc.vector.tensor_scalar_mul(xg1[:], ps[:], invs[ci][:])
                    xg2 = p_xg.tile([128, 512], f32, tag="xg2")
                    nc.vector.scalar_tensor_tensor(
                        xg2[:], ps[:], s2dis[ci][:],
                        xts[ci].rearrange("p b c -> p (b c)")[:, ts(f, 512)],
                        Alu.mult, Alu.subtract)
                    # transposes into the stacks: 8 b's, batched 4 per psum
                    for h in range(2):
                        b0 = f * 8 + h * 4
                        ps_x = p_pst.tile([64, 512], f32, tag="tr")
                        ps_g1 = p_pst.tile([64, 512], f32, tag="tr")
                        ps_g2 = p_pst.tile([64, 512], f32, tag="tr")
                        for q in range(4):
                            bl = h * 4 + q  # b index within this f's 8
                            nc.tensor.transpose(
                                ps_x[:, ts(q, 128)], xts[ci][:, b0 + q, :], ident[:])
                            nc.tensor.transpose(
                                ps_g1[:, ts(q, 128)], xg1[:, ts(bl, 64)], ident[:])
                            nc.tensor.transpose(
                                ps_g2[:, ts(q, 128)], xg2[:, ts(bl, 64)], ident[:])
                        nc.scalar.activation(
                            out=stack1[0:64, b0:b0 + 4, :].rearrange("p b n -> p (b n)"),
                            in_=ps_x[:], func=Act.Copy)
                        nc.vector.tensor_copy(
                            out=stack1[64:128, b0:b0 + 4, :].rearrange("p b n -> p (b n)"),
                            in_=ps_g1[:])
                        nc.vector.tensor_copy(
                            out=stack2[:, b0:b0 + 4, :].rearrange("p b n -> p (b n)"),
                            in_=ps_g2[:])

                # bias transpose: [128 n, 64 o] -> [64 o, 128 n]
                bias_in = p_bt.tile([128, 64], f32, tag="bin")
                nc.scalar.dma_start(out=bias_in, in_=b_d[t, ts(ci, 128), :])
                ps_bt = p_pst.tile([64, 128], f32, tag="tr")
                nc.tensor.transpose(ps_bt[:], bias_in[:], ident[:])
                biasT = p_bt.tile([64, 128], f32, tag="bT")
                nc.vector.tensor_copy(out=biasT[:], in_=ps_bt[:])

                outT = p_ot.tile([64, 32, 128], f32, tag="outT")
                for nn in range(4):  # 32-n weight blocks
                    n0 = nn * 32
                    w01 = p_w.tile([128, 32, 64], bf16, tag="w01")
                    nc.gpsimd.dma_start(
                        out=w01,
                        in_=w_d[t, ci * 128 + n0: ci * 128 + n0 + 32, 0:2]
                        .rearrange("n k i o -> (k i) n o"))
                    w2 = p_w.tile([64, 32, 64], bf16, tag="w2")
                    nc.gpsimd.dma_start(
                        out=w2,
                        in_=w_d[t, ci * 128 + n0: ci * 128 + n0 + 32, 2]
                        .rearrange("n i o -> i n o"))
                    for hh in range(2):
                        ps_o = p_ps.tile([64, 16, 32], f32, tag="e2")
                        for j in range(16):
                            jj = hh * 16 + j
                            nl = n0 + jj
                            nc.tensor.matmul(ps_o[:, j, :], w01[:, jj, :],
                                             stack1[:, :, nl], start=True, stop=False)
                            nc.tensor.matmul(ps_o[:, j, :], w2[:, jj, :],
                                             stack2[:, :, nl], start=False, stop=True)
                        # batched bias add + copy to outT
                        bslice = biasT[:, n0 + hh * 16: n0 + hh * 16 + 16]
                        nc.vector.tensor_tensor(
                            out=outT[:, :, n0 + hh * 16: n0 + hh * 16 + 16],
                            in0=ps_o[:].rearrange("p j b -> p b j"),
                            in1=bslice.unsqueeze(1).broadcast_to([64, 32, 16]),
                            op=Alu.add)

                # transpose back: [64 o, 128 n] slices per b -> [128 n, 64 o]
                out_sb = p_ob.tile([128, 32, 64], f32, tag="osb")
                for g in range(8):  # 4 b's per psum tile
                    ps_q = p_pst.tile([128, 4, 64], f32, tag="tr")
                    for q in range(4):
                        bb = g * 4 + q
                        nc.tensor.transpose(ps_q[:, q, :], outT[:, bb, :],
                                            ident[0:64, 0:64])
                    nc.vector.tensor_copy(
                        out=out_sb[:, g * 4:(g + 1) * 4, :].rearrange("p b o -> p (b o)"),
                        in_=ps_q[:].rearrange("p b o -> p (b o)"))
                nc.sync.dma_start(
                    out=o_d[:, t, ts(ci, 128), :].rearrange("b n o -> n b o"),
                    in_=out_sb[:])

    nc.finalize()
    _CACHE["nc"] = nc
    return nc


def run_spmd(inputs, **kwargs):
    from concourse.bass_utils import run_bass_kernel_spmd

    x = np.ascontiguousarray(inputs["x"], dtype=np.float32)
    emb = np.ascontiguousarray(inputs["dn_embeddings"], dtype=np.float32)
    w = np.ascontiguousarray(inputs["weights_pool"], dtype=np.float32)
    bias = np.ascontiguousarray(inputs["bias_pool"], dtype=np.float32)

    nc = build_bass()
    in_maps = []
    for c in range(NCORES):
        sl = slice(c * T_LOC, (c + 1) * T_LOC)
        in_maps.append({
            "x_sh": np.ascontiguousarray(x[:, sl]),
            "emb_sh": np.ascontiguousarray(emb[sl]),
            "w_sh": np.ascontiguousarray(w[sl]),
            "bias_sh": np.ascontiguousarray(bias[sl]),
        })
    res = run_bass_kernel_spmd(nc, in_maps, core_ids=list(range(NCORES)), **kwargs)
    out = np.concatenate([r["out_sh"] for r in res.results], axis=1)
    return out, res


def kernel(**inputs):
    out, _ = run_spmd(inputs)
    return out

